# revision 19
# baseline (speedup 1.0000x reference)
"""Causal self-attention (B=2, S=2048, D=1024, H=16, Dh=64) on 8 trn2 cores.

Sharding: data-parallel over batch (2 groups of 4 cores) x tensor-parallel over
heads (4 heads/core). Each core computes its heads' attention and a partial
c_proj product; the host sums the 4 partials per batch and adds b_proj plus the
folded V-bias term (softmax weights sum to 1, so bv contributes bv @ W_proj).

v2 changes vs v1:
  - all matmul operands bf16 (FWL weight-load hiding; fp32r disables FWL),
    halves DMA and SBUF; psum stays f32.
  - scores exp split into per-(head, k-tile) halves so the two K=64 row-group
    matmuls (partitions 0:64 / 64:128) stay adjacent in issue order and run
    concurrently on the PE sub-arrays.
  - V projection bias folded into the host epilogue; no ones-column bias matmul.
  - bf16 transposes (1 cyc/row instead of 2).
  - y written bf16; first x/w tiles DMA'd in small chunks so QK starts early.
"""

import os
import sys

for _p in ("/opt/trn_rl_repo", "/root/.axon_site/_ro/trn_rl_repo"):
    if os.path.isdir(_p) and _p not in sys.path:
        sys.path.insert(0, _p)

import numpy as np
import ml_dtypes

import concourse.bacc as bacc
import concourse.tile as tile
from concourse import mybir
from concourse.bass_utils import run_bass_kernel_spmd
from concourse.masks import make_identity

F32 = mybir.dt.float32
BF16 = mybir.dt.bfloat16
FP8 = mybir.dt.float8e4

B, S, D, H, DH = 2, 2048, 1024, 16, 64
HC = 4          # heads per core
EQK = 512       # q+k weight cols per core (2*HC*DH)
EV = 256        # v weight cols per core (HC*DH)
ND = D // 128   # 8 d-tiles
NS = S // 128   # 16 s-tiles (also k-tiles)
NQ = S // 512   # 4 q512-tiles


def build_nc():
    nc = bacc.Bacc("TRN2", target_bir_lowering=False, debug=False)

    xT = nc.dram_tensor("xT", [D, S], BF16, kind="ExternalInput").ap()
    x8 = nc.dram_tensor("x8", [D, S], FP8, kind="ExternalInput").ap()
    wqk8 = nc.dram_tensor("wqk8", [D, EQK], FP8, kind="ExternalInput").ap()
    bqk = nc.dram_tensor("bqk", [128, 4], F32, kind="ExternalInput").ap()
    wv = nc.dram_tensor("wv", [D, EV], BF16, kind="ExternalInput").ap()
    wp = nc.dram_tensor("wp", [EV, D], BF16, kind="ExternalInput").ap()
    masks = nc.dram_tensor("masks", [128, 2, 256], BF16, kind="ExternalInput").ap()
    y = nc.dram_tensor("y", [S, D], BF16, kind="ExternalOutput").ap()

    with tile.TileContext(nc) as tc:
        _emit(nc, tc, xT, x8, wqk8, bqk, wv, wp, masks, y)
    nc.compile()
    return nc


def _emit(nc, tc, xT, x8, wqk8, bqk, wv, wp, masks, y):
    from contextlib import ExitStack

    with ExitStack() as top:
        consts = top.enter_context(tc.tile_pool(name="consts", bufs=1))
        acts = top.enter_context(tc.tile_pool(name="acts", bufs=1))

        identity_f32 = consts.tile([128, 128], F32)
        make_identity(nc, identity_f32)
        identity = consts.tile([128, 128], BF16)
        nc.vector.tensor_copy(identity[:], identity_f32[:])
        masks_sb = consts.tile([128, 2, 256], BF16)
        bqk_sb = consts.tile([128, 4], F32)
        wp_sb = consts.tile([128, 2, D], BF16)

        # persistent activations
        # Q^T/K^T, [dh, S] layout, head-pairs packed on partitions (0:64 / 64:128)
        qt_sb = [acts.tile([128, S], BF16, tag=f"qt{hp}", name=f"qt{hp}") for hp in range(2)]
        kt_sb = [acts.tile([128, S], BF16, tag=f"kt{hp}", name=f"kt{hp}") for hp in range(2)]
        # [V | 1] per head per k-tile: [128, h, kt, 65] bf16
        vhat = acts.tile([128, HC, NS, DH + 1], BF16, tag="vhat")
        # attention output, natural [q, feat] layout, head pairs side by side
        o_sb = acts.tile([128, 2, NS, 128], BF16, tag="o_sb")
        ot_sb = acts.tile([128, 2, S], BF16, tag="ot_sb")  # O^T [feat, s] for proj

        # ---------------- Phase A: QKV projections ----------------
        with ExitStack() as pa:
            xt_pool = pa.enter_context(tc.tile_pool(name="xt", bufs=1))
            w_pool = pa.enter_context(tc.tile_pool(name="w", bufs=1))
            ps_qk = pa.enter_context(tc.tile_pool(name="ps_qk", bufs=5, space="PSUM"))
            ps_v = pa.enter_context(tc.tile_pool(name="ps_v", bufs=3, space="PSUM"))

            xt_sb = xt_pool.tile([128, ND, S], BF16)
            x8_sb = xt_pool.tile([128, ND // 2, 2, S], FP8)
            wqk8_sb = w_pool.tile([128, ND // 2, 2, EQK], FP8)
            wv_sb = w_pool.tile([128, ND, EV], BF16)
            # fp8 QK operands first (small) so the DoubleRow chain starts
            # early; batched multi-tile APs keep the sync-queue trigger count
            # (and its ~0.6us/launch serialization) low.
            nc.sync.dma_start(
                wqk8_sb[:],
                wqk8.rearrange("(dp j p) e -> p dp j e", p=128, j=2))
            for dp in range(ND // 2):
                nc.sync.dma_start(
                    x8_sb[:, dp, :, :],
                    x8[256 * dp:256 * (dp + 1), :].rearrange("(j p) s -> p j s", p=128))
            # bf16 x / wv for the V projection stream in behind the fp8 tiles
            nc.sync.dma_start(
                wv_sb[:], wv.rearrange("(t p) e -> p t e", p=128))
            nc.sync.dma_start(
                xt_sb[:], xT.rearrange("(t p) s -> p t s", p=128))
            # consts are needed later; keep them off the critical DMA path
            nc.sync.dma_start(bqk_sb[:], bqk[:])
            nc.sync.dma_start(masks_sb[:], masks[:])
            for t in range(2):
                nc.sync.dma_start(wp_sb[:, t, :], wp[128 * t:128 * (t + 1), :])

            # Q^T/K^T: psum[e128, s512] = sum_d wqk[d,e].T @ xT[d,s], fp8
            # DoubleRow: each matmul contracts a 256-row d-pair at 2 rows/cyc.
            # e-tile order: 0 -> Q hp0, 1 -> K hp0, 2 -> Q hp1, 3 -> K hp1
            # dtp is the OUTER loop so compute starts when the first pair lands
            for et in range(4):
                dest = (qt_sb if et % 2 == 0 else kt_sb)[et // 2]
                ps = [ps_qk.tile([128, 512], F32, tag="pqk", name=f"pqk{st}") for st in range(NQ)]
                for dtp in range(ND // 2):
                    for st in range(NQ):
                        nc.tensor.matmul(
                            ps[st][:],
                            wqk8_sb[:, dtp, :, 128 * et:128 * (et + 1)],
                            x8_sb[:, dtp, :, 512 * st:512 * (st + 1)],
                            start=(dtp == 0), stop=(dtp == ND // 2 - 1),
                            perf_mode=mybir.MatmulPerfMode.DoubleRow,
                        )
                for st in range(NQ):
                    nc.vector.tensor_scalar_add(
                        dest[:, 512 * st:512 * (st + 1)], ps[st][:], bqk_sb[:, et:et + 1]
                    )

            # V natural: psum[s128, 256] = sum_d xT[d,s].T @ wv[d,:]
            # (bias folded into the host epilogue: softmax weights sum to 1)
            for st in range(NS):
                p = ps_v.tile([128, EV], F32)
                for dt in range(ND):
                    nc.tensor.matmul(
                        p[:],
                        xt_sb[:, dt, 128 * st:128 * (st + 1)],
                        wv_sb[:, dt, :],
                        start=(dt == 0), stop=(dt == ND - 1),
                    )
                nc.any.tensor_copy(
                    vhat[:, :, st, 0:DH],
                    p[:].rearrange("p (h e) -> p h e", h=HC),
                )
            nc.vector.memset(vhat[:, :, :, DH:DH + 1], 1.0)

        # ---------------- Phase B: attention (q256 blocks) ----------------
        # scores^T/exp for iteration i are interleaved with the PV/transpose
        # work of iteration i-1 so PE keeps streaming while ScalarE runs exp.
        # q-blocks of 256 cut the block-causal overshoot to 6%, and the
        # single-bank psc tiles (bufs=4) push the WAR dependency two groups
        # back so each group's four K=64 scores matmuls become ready together
        # and the h0/h64 row-group pairs run concurrently.
        NJ = S // 256
        with ExitStack() as pb:
            ps_sc = pb.enter_context(tc.tile_pool(name="ps_sc", bufs=4, space="PSUM"))
            ps_ot = pb.enter_context(tc.tile_pool(name="ps_ot", bufs=1, space="PSUM"))
            ps_tp = pb.enter_context(tc.tile_pool(name="ps_tp", bufs=1, space="PSUM"))
            ps_y = pb.enter_context(tc.tile_pool(name="ps_y", bufs=2, space="PSUM"))
            outp = pb.enter_context(tc.tile_pool(name="outp", bufs=4))
            phat_pool = pb.enter_context(tc.tile_pool(name="phat", bufs=2))
            otsb_pool = pb.enter_context(tc.tile_pool(name="otsb", bufs=3))
            small = pb.enter_context(tc.tile_pool(name="small", bufs=8))

            def emit_pv(hp, jq, phat):
                """PV (streaming, lhsT=V stationary) + transpose-back + normalize.
                out^T[65, q256] = sum_kt vhat[:,h,kt,:].T @ phat[kt, q]; row 64
                is the softmax denominator. Transpose 128-blocks back to
                [q, 65], then reciprocal-scale into o_sb. Yields between
                instructions so the caller can interleave emission."""
                nk = 2 * (jq + 1)
                for h in range(2):
                    hg = 2 * hp + h
                    po = ps_ot.tile([65, 256], F32, tag="ot", name="po")
                    for kt in range(nk):
                        nc.tensor.matmul(
                            po[:],
                            vhat[:, hg, kt, :],
                            phat[h][:, 256 * kt:256 * (kt + 1)],
                            start=(kt == 0), stop=(kt == nk - 1),
                        )
                        yield
                    so = otsb_pool.tile([65, 256], BF16, tag="so", name="so")
                    nc.any.tensor_copy(so[:], po[:])
                    yield
                    for b2 in range(2):
                        jq2 = 2 * jq + b2
                        pt = ps_tp.tile([128, DH + 1], BF16, tag="tp", name="pt")
                        nc.tensor.transpose(
                            pt[:], so[:, 128 * b2:128 * (b2 + 1)], identity[0:65, 0:65]
                        )
                        recip = small.tile([128, 1], F32)
                        nc.vector.reciprocal(recip[:], pt[:, DH:DH + 1])
                        nc.vector.tensor_scalar_mul(
                            o_sb[:, hp, jq2, 64 * h:64 * (h + 1)], pt[:, 0:DH], recip[:]
                        )
                        yield
                # both heads normalized: feed the proj-prep transposes now so
                # the kernel tail is just the proj matmuls
                for b2 in range(2):
                    jq2 = 2 * jq + b2
                    pt2 = ps_tp.tile([128, 128], BF16, tag="tp", name="pt2")
                    nc.tensor.transpose(pt2[:], o_sb[:, hp, jq2, :], identity[:])
                    nc.any.tensor_copy(ot_sb[:, hp, 128 * jq2:128 * (jq2 + 1)], pt2[:])
                    yield
                if hp == 1:
                    # both feature halves of ot_sb are now final for these s
                    # tiles: emit their slice of the projection
                    for b2 in range(2):
                        st = 2 * jq + b2
                        for nt in range(2):
                            py = ps_y.tile([128, 512], F32, tag="py", name="py")
                            for ft in range(2):
                                nc.tensor.matmul(
                                    py[:],
                                    ot_sb[:, ft, 128 * st:128 * (st + 1)],
                                    wp_sb[:, ft, 512 * nt:512 * (nt + 1)],
                                    start=(ft == 0), stop=(ft == 1),
                                )
                            ys = outp.tile([128, 512], BF16, tag="ys", name="ys")
                            nc.any.tensor_copy(ys[:], py[:])
                            nc.sync.dma_start(
                                y[128 * st:128 * (st + 1), 512 * nt:512 * (nt + 1)], ys[:]
                            )
                            yield

            prev = iter(())
            prev_len = 0
            for hp in range(2):
                for jq in range(NJ):
                    nk = 2 * (jq + 1)  # k-tiles in causal range
                    ngrp = nk // 2
                    phat = [phat_pool.tile([128, NS * 256], BF16, tag=f"phat{h}", name=f"phat{h}") for h in range(2)]
                    nchunk = prev_len // ngrp + 3
                    for g in range(ngrp):
                        psc = [ps_sc.tile([128, 512], F32, tag="sc", name=f"sc{h}") for h in range(2)]
                        for j in range(2):
                            kt = 2 * g + j
                            for h in range(2):
                                sl = slice(64 * h, 64 * (h + 1))
                                nc.tensor.matmul(
                                    psc[h][:, 256 * j:256 * (j + 1)],
                                    kt_sb[hp][sl, 128 * kt:128 * (kt + 1)],
                                    qt_sb[hp][sl, 256 * jq:256 * (jq + 1)],
                                    start=True, stop=True,
                                )
                        for h in range(2):
                            nc.scalar.activation(
                                phat[h][:, 512 * g:512 * (g + 1)],
                                psc[h][:],
                                mybir.ActivationFunctionType.Exp,
                                # q,k carry a 16x scale (host) to keep the fp8
                                # weights out of e4m3's subnormal range
                                scale=0.125 / 256.0,
                            )
                            # causal mask on the two diagonal k-tiles (always
                            # the last group of the block)
                            if g == ngrp - 1:
                                for j in range(2):
                                    kt = 2 * g + j
                                    sl = slice(256 * kt, 256 * (kt + 1))
                                    nc.vector.tensor_mul(
                                        phat[h][:, sl], phat[h][:, sl],
                                        masks_sb[:, j, :],
                                    )
                        # interleave a slice of the previous iteration's PV work
                        for _ in range(nchunk):
                            if next(prev, None) is None:
                                break
                    for _ in prev:
                        pass  # drain any leftover PV work before swapping
                    prev = emit_pv(hp, jq, phat)
                    prev_len = 2 * nk + 8 + (4 if hp == 1 else 0)
            for _ in prev:
                pass


_NC = None


def _get_nc():
    global _NC
    if _NC is None:
        _NC = build_nc()
    return _NC


def _make_masks():
    i = np.arange(128)[:, None]
    j = np.arange(256)[None, :]
    m = np.stack([(i + 128 * o <= j) for o in range(2)], axis=1)  # [128, 2, 256]
    return m.astype(ml_dtypes.bfloat16)


def _in_maps(x, W_attn, b_attn, W_proj):
    bf = ml_dtypes.bfloat16
    f8 = ml_dtypes.float8_e4m3
    masks = _make_masks()
    maps = []
    for c in range(8):
        b, g = c // 4, c % 4
        heads = [4 * g + i for i in range(HC)]
        qc = [W_attn[:, 64 * h:64 * (h + 1)] for h in heads]
        kc = [W_attn[:, D + 64 * h:D + 64 * (h + 1)] for h in heads]
        vc = [W_attn[:, 2 * D + 64 * h:2 * D + 64 * (h + 1)] for h in heads]
        bq = [b_attn[64 * h:64 * (h + 1)] for h in heads]
        bk = [b_attn[D + 64 * h:D + 64 * (h + 1)] for h in heads]
        wqk_c = np.ascontiguousarray(np.concatenate(
            [qc[0], qc[1], kc[0], kc[1], qc[2], qc[3], kc[2], kc[3]], axis=1))
        bqk_c = np.concatenate(
            [bq[0], bq[1], bk[0], bk[1], bq[2], bq[3], bk[2], bk[3]])
        bqk_c = np.ascontiguousarray(bqk_c.reshape(4, 128).T)
        wv_c = np.ascontiguousarray(np.concatenate(vc, axis=1))
        wp_c = np.ascontiguousarray(W_proj[EV * g:EV * (g + 1), :])
        xT_c = np.ascontiguousarray(x[b].T)
        maps.append({
            "xT": xT_c.astype(bf), "x8": xT_c.astype(f8),
            # 16x scale keeps the uniform(+-1/32) weights in e4m3's normal
            # range; undone by the exp scale (q and k each carry 16x)
            "wqk8": (wqk_c * 16.0).astype(f8), "bqk": bqk_c * 16.0,
            "wv": wv_c.astype(bf), "wp": wp_c.astype(bf), "masks": masks,
        })
    return maps


def _gather(results, b_attn, W_proj, b_proj):
    # softmax weights sum to 1, so the V bias contributes exactly bv @ W_proj
    yb = b_proj + b_attn[2 * D:3 * D].astype(np.float64) @ W_proj.astype(np.float64)
    y = np.empty((B, S, D), np.float32)
    for b in range(B):
        acc = results[4 * b]["y"].astype(np.float32)
        for g in range(1, 4):
            acc = acc + results[4 * b + g]["y"].astype(np.float32)
        y[b] = acc + yb.astype(np.float32)[None, :]
    return y


def run(x, W_attn, b_attn, W_proj, b_proj, trace=False):
    x = np.asarray(x, np.float32)
    W_attn = np.asarray(W_attn, np.float32)
    b_attn = np.asarray(b_attn, np.float32)
    W_proj = np.asarray(W_proj, np.float32)
    b_proj = np.asarray(b_proj, np.float32)
    nc = _get_nc()
    res = run_bass_kernel_spmd(nc, _in_maps(x, W_attn, b_attn, W_proj),
                               core_ids=list(range(8)), trace=trace)
    return _gather(res.results, b_attn, W_proj, b_proj), res


def kernel(x, W_attn, b_attn, W_proj, b_proj):
    out, _ = run(x, W_attn, b_attn, W_proj, b_proj)
    return out
